# revision 1
# baseline (speedup 1.0000x reference)
"""AutoWeightedCELoss Trainium2 kernel.

Computes mean(class_w[label] * CE(cls_score, label) * boundary_weight) for
B=8, C=4, H=W=512, data-parallel over 8 NeuronCores (1 sample per core).

Math (per sample):
  boundary weight: out(x) = sum_k box_k(1 - onehot_{l(x)})(x) / (k^2-1), k=3,5,9,17,33
  With the label's 2 bits encoded as +-1 "spin" maps sa, sb, sab=sa*sb:
    box_k(onehot_l)(x) = 1/4 [A_k + sa(x) Bk(sa)(x) + sb(x) Bk(sb)(x) + sab(x) Bk(sab)(x)]
  so pix_w = CON + sa*Ga + sb*Gb + sab*Gab, where
    CON   = 1 + 0.75 * sum_k A_k/(k^2-1)          (position-only, host-precomputed)
    G_m   = sum_k c'_k box_k(m), c'_k = -1/(4(k^2-1))   (3 maps)

  box computation (all in transposed (w,h) layout so shifts ride the free axis):
    Cv^T[w,h'] = sum_h m[h,w] U[h,h']          PE matmul, triangular U, fp16
    Dv_k[w,h'] = Cv^T(:,h'+p) - Cv^T(:,h'-p-1) DVE f16 shift-diff (vert box, <=k, exact)
    G^T[w',h'] = sum_k sum_w (c'_k M_k)[w,w'] Dv_k[w,h']   PE matmuls, band M_k,
                 all 15 (scale,chunk) groups PSUM-accumulated per map
  The only inexactness vs fp32 reference: c'_k rounded to fp16 (rel ~2^-11).

  CE: nll = log(sum_c exp(s_c)) - s_label  (scores are N(0,1): no max needed).
  The CE tail runs in transposed layout too (nll is PE-transposed, off the
  critical path), so pix^T never needs transposing back.
  Class sums via spin algebra: S_c = 1/4 (T0 + sig_a(c) Ta + sig_b(c) Tb
  + sig_ab(c) Tab), T* = per-partition reductions of q=nll*pix_w against spin
  maps; n_c likewise. Reduced on host; loss = sum_c W_c S_c / N.
"""

import sys

sys.path.insert(0, "/opt/trn_rl_repo")

import numpy as np

import concourse.bacc as bacc
import concourse.mybir as mybir
from concourse import bass
from concourse.tile import TileContext
from concourse.bass_utils import run_bass_kernel_spmd

F32 = mybir.dt.float32
F16 = mybir.dt.float16
I32 = mybir.dt.int32
I8 = mybir.dt.int8
OP = mybir.AluOpType
ACTF = mybir.ActivationFunctionType

B, C, H, W = 8, 4, 512, 512
P = 128          # partitions
NT = H // P      # 4 h-tiles (and w-tiles)
WID = NT * W     # 2048 wide-tile free size
N_CORES = 8
KS = [3, 5, 9, 17, 33]
PADS = [1, 2, 4, 8, 16]
CP = [-1.0 / (4.0 * (k * k - 1)) for k in KS]   # -c_k/4


def _host_constants():
    h = np.arange(H, dtype=np.float64)
    U = np.triu(np.ones((H, H), dtype=np.float16))            # U[h,h'] = h<=h'
    M = np.zeros((len(KS), W, W), dtype=np.float16)
    con = np.ones((H, W), dtype=np.float64)
    for i, k in enumerate(KS):
        p = PADS[i]
        d = np.abs(np.arange(W)[:, None] - np.arange(W)[None, :])
        M[i] = (d <= p).astype(np.float16) * np.float16(CP[i])
        rc = np.minimum(h + p, H - 1) - np.maximum(h - p, 0) + 1  # rows in window
        A = rc[:, None] * rc[None, :]
        con += 0.75 * A / (k * k - 1)
    return U, M, con.astype(np.float32)


def _wide(dram_ap):
    """(H, W) dram tensor -> [P, NT, W] access pattern (h-tiles stacked)."""
    return dram_ap.rearrange("(t p) w -> p t w", p=P)


def _w3(tile_ap):
    """[P, NT*W] sbuf tile -> [P, NT, W] view to pair with _wide()."""
    return tile_ap.rearrange("p (t w) -> p t w", t=NT)


def _dma_split(nc, tile, dram, nsplit, eng=None):
    """DMA a (H,W)-style dram tensor into a wide tile as `nsplit` separate
    transfers (different queues) to beat the ~22GB/s per-queue limit."""
    eng = eng or nc.sync
    step = NT // nsplit if nsplit <= NT else None
    if step:  # split by h-tile blocks
        for s in range(nsplit):
            t0 = s * step
            eng.dma_start(
                _w3(tile[:])[:, t0 : t0 + step, :],
                _wide(dram)[:, t0 : t0 + step, :],
            )
    else:  # split each h-tile block in half along w
        for t0 in range(NT):
            for half in range(2):
                eng.dma_start(
                    _w3(tile[:])[:, t0 : t0 + 1, half * (W // 2) : (half + 1) * (W // 2)],
                    _wide(dram)[:, t0 : t0 + 1, half * (W // 2) : (half + 1) * (W // 2)],
                )


def build_nc(debug=False):
    nc = bacc.Bacc(None, target_bir_lowering=False, debug=True)

    score = nc.dram_tensor("score", [C, H, W], F32, kind="ExternalInput")
    label = nc.dram_tensor("label", [H, W], I32, kind="ExternalInput")
    u16d = nc.dram_tensor("u16", [H, H], F16, kind="ExternalInput")
    m16d = nc.dram_tensor("m16", [len(KS), W, W], F16, kind="ExternalInput")
    cond = nc.dram_tensor("con", [H, W], F32, kind="ExternalInput")
    eyed = nc.dram_tensor("eye", [P, P], F32, kind="ExternalInput")
    eye16d = nc.dram_tensor("eye16", [P, P], F16, kind="ExternalInput")
    # reduction partials: [T0, Ta, Tb, Tab] and [Na, Nb, Nab]
    t_part = nc.dram_tensor("t_part", [P, 4], F32, kind="ExternalOutput")
    n_part = nc.dram_tensor("n_part", [P, 3], F32, kind="ExternalOutput")
    if debug:
        pix_dbg = nc.dram_tensor("pix_dbg", [W, H], F32, kind="ExternalOutput")
        nll_dbg = nc.dram_tensor("nll_dbg", [H, W], F32, kind="ExternalOutput")
        gt_dbg = nc.dram_tensor("gt_dbg", [3, W, H], F32, kind="ExternalOutput")

    with TileContext(nc) as tc:
        with (
            tc.tile_pool(name="sb", bufs=1) as sb,
            tc.tile_pool(name="ps", bufs=1, space="PSUM") as ps,
        ):
            # ---- label first: it gates the whole G pipeline ----
            lbl_i = sb.tile([P, WID], I32, tag="lbl_i")
            _dma_split(nc, lbl_i, label[:], 8)

            # spins straight from the int tile (ALU converts per-operand)
            a_m = sb.tile([P, WID], F32, tag="scr_a")   # bit1 = [l>=2]
            b_m = sb.tile([P, WID], F32, tag="scr_b")   # bit0 = l - 2*bit1
            nc.vector.tensor_scalar(a_m[:], lbl_i[:], 2.0, None, OP.is_ge)
            nc.vector.scalar_tensor_tensor(
                b_m[:], a_m[:], -2.0, lbl_i[:], OP.mult, OP.add
            )
            spins = []
            for nm, src in (("sa", a_m), ("sb", b_m)):
                t = sb.tile([P, WID], F16, tag=nm)
                nc.vector.tensor_scalar(t[:], src[:], -2.0, 1.0, OP.mult, OP.add)
                spins.append(t)
            sab = sb.tile([P, WID], F16, tag="sab")
            nc.vector.tensor_mul(sab[:], spins[0][:], spins[1][:])
            spins.append(sab)

            # ---- constants (u16 needed first, m16 by pass2, con by combine) --
            u16 = sb.tile([P, WID], F16, tag="u16")
            _dma_split(nc, u16, u16d[:], 4)
            eye16 = sb.tile([P, P], F16, tag="eye16")
            nc.sync.dma_start(eye16[:], eye16d[:])
            eye = sb.tile([P, P], F32, tag="eye")
            nc.sync.dma_start(eye[:], eyed[:])
            m16 = []
            for i in range(len(KS)):
                t = sb.tile([P, WID], F16, tag=f"m16_{i}")
                _dma_split(nc, t, m16d[i], 4)
                m16.append(t)
            pixt = sb.tile([P, WID], F32, tag="pixt")
            _dma_split(nc, pixt, cond[:], 4)

            # ---- pass1: Cv^T[w, h'] = sum_h spin[h,w] U[h,h'] (per map) ----
            # U upper-triangular: contraction chunk tt only reaches h' >= 128tt.
            cvt = []
            for mi, sp in enumerate(spins):
                t = sb.tile([P, WID], F16, tag=f"cvt_{mi}")
                for j in range(NT):  # w-chunk -> psum partitions
                    pst = ps.tile([P, W], F32, tag="ps_cv", bufs=2)
                    for tt in range(NT):  # contraction over h-tiles
                        nc.tensor.matmul(
                            pst[:, P * tt : W],
                            sp[:, W * tt + P * j : W * tt + P * j + P],
                            u16[:, W * tt + P * tt : W * tt + W],
                            start=(tt == 0),
                            stop=(tt == NT - 1),
                            skip_group_check=True,
                        )
                    nc.scalar.copy(t[:, bass.ts(j, W)], pst[:])
                cvt.append(t)

            # ---- transposed spins (combine + CE tail run in (w,h) layout) ----
            spins_t = []
            for mi, sp in enumerate(spins):
                t = sb.tile([P, WID], F16, tag=f"spT_{mi}")
                for tw in range(NT):
                    pst = ps.tile([P, W], F16, tag="ps_spT", bufs=2)
                    for th in range(NT):
                        nc.tensor.transpose(
                            pst[:, bass.ts(th, P)],
                            sp[:, W * th + P * tw : W * th + P * tw + P],
                            eye16[:],
                        )
                    nc.scalar.copy(t[:, bass.ts(tw, W)], pst[:])
                spins_t.append(t)

            # Na/Nb/Nab = per-partition sums of the spin maps
            ncols = []
            junk16 = sb.tile([P, WID], F16, tag="junk16")
            for mi, nm in enumerate(("sa", "sb", "sab")):
                col = sb.tile([P, 1], F32, tag=f"ncol_{nm}")
                nc.vector.memset(col[:], 0.0)
                nc.vector.tensor_scalar(
                    junk16[:], spins[mi][:], 1.0, None, OP.mult, OP.add,
                    accum_out=col[:],
                )
                ncols.append(col)

            # ---- per map: Dv shift-diffs (DVE f16) + band matmuls into PSUM --
            gt_tiles = []
            for mi in range(3):
                cv3 = _w3(cvt[mi][:])

                def _build_dv(ki):
                    p = PADS[ki]
                    dv = sb.tile([P, WID], F16, tag="dv", bufs=3)
                    dv3 = _w3(dv[:])
                    # middle: h' in [p+1, W-p)
                    nc.vector.tensor_sub(
                        dv3[:, :, p + 1 : W - p],
                        cv3[:, :, 2 * p + 1 : W],
                        cv3[:, :, 0 : W - 2 * p - 1],
                    )
                    # left edge: h' in [0, p+1): Dv = Cv(h'+p)
                    nc.vector.tensor_copy(
                        dv3[:, :, 0 : p + 1], cv3[:, :, p : 2 * p + 1]
                    )
                    # right edge: h' in [W-p, W): Dv = Cv(511) - Cv(h'-p-1)
                    col = cv3[:, :, W - 1 : W].broadcast_to([P, NT, p])
                    nc.vector.scalar_tensor_tensor(
                        dv3[:, :, W - p : W],
                        cv3[:, :, W - 2 * p - 1 : W - p - 1],
                        -1.0,
                        col,
                        OP.mult,
                        OP.add,
                    )
                    return dv

                gt = sb.tile([P, WID], F32, tag=f"gt_{mi}")
                gps = ps.tile([P, WID], F32, tag="ps_g", bufs=1)
                # scale-major so each dv is consumed before the next is built;
                # start/stop bracket each w'-chunk's PSUM accumulation group.
                for ki in range(len(KS)):
                    p = PADS[ki]
                    dv = _build_dv(ki)
                    for j in range(NT):  # w'-chunk
                        # contraction chunks reaching chunk j: j-1, j, j+1
                        for tt in (j - 1, j, j + 1):
                            if tt < 0 or tt >= NT:
                                continue
                            # partial chunks: band entries outside the needed
                            # rows are zero; align to the PE's 0/32/64 base
                            # partition requirement
                            if tt == j:
                                lo, hi = 0, P
                            elif tt == j - 1:
                                lo, hi = P - 64, P
                            else:
                                lo, hi = 0, 32
                            nc.tensor.matmul(
                                gps[:, bass.ts(j, W)],
                                m16[ki][lo:hi, W * tt + P * j : W * tt + P * j + P],
                                dv[lo:hi, bass.ts(tt, W)],
                                start=(ki == 0 and tt == max(j - 1, 0)),
                                stop=(
                                    ki == len(KS) - 1
                                    and tt == min(j + 1, NT - 1)
                                ),
                                skip_group_check=True,
                            )
                nc.scalar.copy(gt[:], gps[:])
                if debug:
                    nc.sync.dma_start(_wide(gt_dbg[mi]), _w3(gt[:]))
                # combine contribution as soon as this map's G is out
                nc.vector.tensor_mul(gt[:], spins_t[mi][:], gt[:])
                gt_tiles.append(gt)

            # pix^T = CON + v0 + v1 + v2   (stays transposed in pixt)
            nc.gpsimd.tensor_add(gt_tiles[0][:], gt_tiles[0][:], gt_tiles[1][:])
            nc.vector.tensor_add(gt_tiles[2][:], gt_tiles[2][:], pixt[:])
            nc.vector.tensor_add(pixt[:], gt_tiles[0][:], gt_tiles[2][:])
            if debug:
                nc.sync.dma_start(
                    pix_dbg[:].rearrange("(t p) h -> p t h", p=P), _w3(pixt[:])
                )

            # ---- CE (parallel track; only q needs pix^T) ----
            sc = []
            for c in range(C):
                t = sb.tile([P, WID], F32, tag=f"s{c}")
                _dma_split(nc, t, score[c], 4, eng=nc.gpsimd)
                sc.append(t)

            # gather s_label BEFORE overwriting scores with exp
            sl = sb.tile([P, WID], F32, tag="scr_sl")
            nc.vector.tensor_copy(sl[:], sc[0][:])
            for c in range(1, C):
                ohi = sb.tile([P, WID], I8, tag="ohi_scr")
                nc.vector.tensor_scalar(ohi[:], lbl_i[:], float(c), None, OP.is_equal)
                nc.vector.copy_predicated(sl[:], ohi[:], sc[c][:])

            # exp in place, sum, log
            for c in range(C):
                nc.scalar.activation(sc[c][:], sc[c][:], ACTF.Exp)
            nc.vector.tensor_add(sc[0][:], sc[0][:], sc[1][:])
            nc.gpsimd.tensor_add(sc[2][:], sc[2][:], sc[3][:])
            nc.vector.tensor_add(sc[0][:], sc[0][:], sc[2][:])
            lse = sc[1]  # reuse
            nc.scalar.activation(lse[:], sc[0][:], ACTF.Ln)

            # nll = lse - s_l, then transpose to (w,h) via PE
            nll = sc[3]  # reuse
            nc.vector.tensor_sub(nll[:], lse[:], sl[:])
            if debug:
                nc.sync.dma_start(_wide(nll_dbg[:]), _w3(nll[:]))
            nllt = sc[2]  # reuse
            for tw in range(NT):
                pst = ps.tile([P, W], F32, tag="ps_cv", bufs=2)
                for th in range(NT):
                    nc.tensor.transpose(
                        pst[:, bass.ts(th, P)],
                        nll[:, W * th + P * tw : W * th + P * tw + P],
                        eye[:],
                    )
                nc.scalar.copy(nllt[:, bass.ts(tw, W)], pst[:])

            # q^T = nll^T * pix^T with T0 accumulation; then Ta/Tb/Tab
            q_m = sc[0]  # reuse
            t0_col = sb.tile([P, 1], F32, tag="t0col")
            nc.vector.memset(t0_col[:], 0.0)
            nc.vector.scalar_tensor_tensor(
                q_m[:], nllt[:], 1.0, pixt[:], OP.mult, OP.mult,
                accum_out=t0_col[:],
            )
            nc.sync.dma_start(t_part[:, 0:1], t0_col[:])
            junk = sc[1]  # reuse
            for mi in range(3):
                tcol = sb.tile([P, 1], F32, tag=f"tcol{mi}")
                nc.vector.memset(tcol[:], 0.0)
                nc.vector.scalar_tensor_tensor(
                    junk[:], q_m[:], 1.0, spins_t[mi][:], OP.mult, OP.mult,
                    accum_out=tcol[:],
                )
                nc.sync.dma_start(t_part[:, mi + 1 : mi + 2], tcol[:])
                nc.sync.dma_start(n_part[:, mi : mi + 1], ncols[mi][:])

    nc.finalize()
    return nc


_CACHE = {}


def _get_nc(debug=False):
    key = "dbg" if debug else "fast"
    if key not in _CACHE:
        _CACHE[key] = build_nc(debug)
    return _CACHE[key]


def run_cores(cls_score, label, debug=False, trace=False):
    """Run the SPMD kernel; returns BassKernelResults."""
    U, M, CON = _host_constants()
    eye = np.eye(P, dtype=np.float32)
    eye16 = np.eye(P, dtype=np.float16)
    in_maps = []
    for i in range(N_CORES):
        in_maps.append(
            {
                "score": np.ascontiguousarray(cls_score[i]),
                "label": np.ascontiguousarray(label[i]),
                "u16": U,
                "m16": M,
                "con": CON,
                "eye": eye,
                "eye16": eye16,
            }
        )
    nc = _get_nc(debug)
    return run_bass_kernel_spmd(nc, in_maps, list(range(N_CORES)), trace=trace)


def kernel(cls_score, label):
    cls_score = np.asarray(cls_score, dtype=np.float32)
    label = np.asarray(label, dtype=np.int32)
    res = run_cores(cls_score, label)
    # T0, Ta, Tb, Tab and Na, Nb, Nab summed over cores+partitions
    T = np.zeros(4, dtype=np.float64)
    N = np.zeros(3, dtype=np.float64)
    for r in res.results:
        T += r["t_part"].astype(np.float64).sum(axis=0)
        N += r["n_part"].astype(np.float64).sum(axis=0)
    npix = float(B * H * W)
    loss = 0.0
    for c in range(C):
        sig_a = 1.0 - 2.0 * (c >> 1)
        sig_b = 1.0 - 2.0 * (c & 1)
        n_c = 0.25 * (npix + sig_a * N[0] + sig_b * N[1] + sig_a * sig_b * N[2])
        s_c = 0.25 * (T[0] + sig_a * T[1] + sig_b * T[2] + sig_a * sig_b * T[3])
        w_c = 2.0 / (n_c / npix + 1.0)
        loss += w_c * s_c
    return np.float32(loss / npix)


if __name__ == "__main__":
    rng = np.random.default_rng(0)
    cs = rng.standard_normal((B, C, H, W)).astype(np.float32)
    lb = rng.integers(0, C, size=(B, H, W)).astype(np.int32)
    print("loss:", kernel(cs, lb))

